# revision 12
# baseline (speedup 1.0000x reference)
"""Trainium2 Bass kernel for nn_LiquidNet2 (liquid time-constant ODE unfolds).

Device strategy (unchanged from the working baseline): shard the postsynaptic
dim S=512 across 8 cores (64 neurons each), keep the full batch B=1024 on
every core so ACT instructions run with free dim 1024.  Each unfold:
  - ACT: s = sigmoid(sigma[j,k] * v[j,b] - sigma*mu[j,k]) per (j-tile, k),
    fused affine via per-partition scale/bias APs.            [256 instrs]
  - PE : per (k, j-tile, b-subtile) matmul with the sigmoid tile stationary
    and the fp16 [W*erev | W] column pair moving; (num,den) accumulate at
    free offset 2k of a shared PSUM tile (partition = batch). [2048 matmuls]
  - DVE: v_new = (cm*v + rec_num + sens_num') / (rec_den + sens_den')
    on [128 batch, 64 k] tiles (sens' has gleak*vleak / cm+gleak folded in).
  - PE transpose v_new -> [64 k, 1024 b], AllGather across cores -> next
    unfold's [512, 1024] state (skipped after the last unfold).
The sensory (input synapse) pass has the same structure with I=128 as the
presynaptic dim and runs once, staying resident as PSUM->SBUF tiles.

Host strategy (new): the SPMD program is compiled ONCE into a reusable
jax/PJRT executable (shard_map over the 8 cores).  Inputs are packed and
uploaded to the devices once and reused across calls while their content is
unchanged (checked with np.array_equal).  The donated output-init buffer is
the previous call's output buffer, so a steady-state invocation is exactly
one executable dispatch plus one result fetch.
"""

import numpy as np

B, I, S = 1024, 128, 512
UNFOLDS = 6
NCORES = 8
KLOC = S // NCORES      # 64 postsynaptic neurons per core
NJT = S // 128          # 4 presynaptic j-tiles
NBS = B // 128          # 8 batch subtiles

_CACHE = {}


CHUNKS = (32, 32)   # split of the 64 local k per unfold; the AllGather of
                    # every chunk but the last overlaps the next chunk's ACT


def _build_program(unfolds=UNFOLDS):
    import concourse.bacc as bacc
    import concourse.tile as tile
    import concourse.mybir as mybir
    from contextlib import ExitStack

    dt = mybir.dt
    AF = mybir.ActivationFunctionType
    f32, f16 = dt.float32, dt.float16

    nc = bacc.Bacc("TRN2", target_bir_lowering=False, debug=False,
                   num_devices=NCORES)

    xT_d = nc.dram_tensor("xT", [I, B], f32, kind="ExternalInput")
    id_d = nc.dram_tensor("ident", [128, 128], f32, kind="ExternalInput")
    hxT_d = nc.dram_tensor("hxT", [S, B], f16, kind="ExternalInput")
    hxbk_d = nc.dram_tensor("hxbk", [B, KLOC], f32, kind="ExternalInput")
    rsc_d = nc.dram_tensor("rsc", [128, NJT * KLOC], f32, kind="ExternalInput")
    rbi_d = nc.dram_tensor("rbi", [128, NJT * KLOC], f32, kind="ExternalInput")
    rwp_d = nc.dram_tensor("rwp", [128, NJT * KLOC * 2], f16, kind="ExternalInput")
    ssc_d = nc.dram_tensor("ssc", [I, KLOC], f32, kind="ExternalInput")
    sbi_d = nc.dram_tensor("sbi", [I, KLOC], f32, kind="ExternalInput")
    swp_d = nc.dram_tensor("swp", [I, KLOC * 2], f16, kind="ExternalInput")
    cm_d = nc.dram_tensor("cmr", [128, KLOC], f32, kind="ExternalInput")
    gvl_d = nc.dram_tensor("gvlr", [128, KLOC], f32, kind="ExternalInput")
    cg_d = nc.dram_tensor("cgr", [128, KLOC], f32, kind="ExternalInput")
    out_d = nc.dram_tensor("out", [B, KLOC], f32, kind="ExternalOutput")

    with tile.TileContext(nc) as tc, ExitStack() as ctx:
        const = ctx.enter_context(tc.tile_pool(name="const", bufs=1))
        vt_pool = ctx.enter_context(tc.tile_pool(name="vt", bufs=2))
        s_pool = ctx.enter_context(tc.tile_pool(name="sig", bufs=4))
        vbk_pool = ctx.enter_context(tc.tile_pool(name="vbk", bufs=2))
        upd_pool = ctx.enter_context(tc.tile_pool(name="upd", bufs=2))
        vloc_pool = ctx.enter_context(tc.tile_pool(name="vloc", bufs=2))
        sens_pool = ctx.enter_context(tc.tile_pool(name="sens", bufs=1))
        ps_rec = ctx.enter_context(tc.tile_pool(name="psr", bufs=2, space="PSUM"))
        ps_sens = ctx.enter_context(tc.tile_pool(name="pss", bufs=1, space="PSUM"))
        ps_tr = ctx.enter_context(tc.tile_pool(name="pst", bufs=1, space="PSUM"))
        dram = ctx.enter_context(tc.tile_pool(name="dram", bufs=2, space="DRAM"))

        # ---- resident inputs ----
        xT = const.tile([I, B], f32)
        nc.sync.dma_start(xT[:], xT_d[:])
        ident = const.tile([128, 128], f32)
        nc.sync.dma_start(ident[:], id_d[:])
        rsc = const.tile([128, NJT * KLOC], f32)
        nc.sync.dma_start(rsc[:], rsc_d[:])
        rbi = const.tile([128, NJT * KLOC], f32)
        nc.sync.dma_start(rbi[:], rbi_d[:])
        rwp = const.tile([128, NJT * KLOC * 2], f16)
        nc.sync.dma_start(rwp[:], rwp_d[:])
        ssc = const.tile([I, KLOC], f32)
        nc.sync.dma_start(ssc[:], ssc_d[:])
        sbi = const.tile([I, KLOC], f32)
        nc.sync.dma_start(sbi[:], sbi_d[:])
        swp = const.tile([I, KLOC * 2], f16)
        nc.sync.dma_start(swp[:], swp_d[:])
        cm = const.tile([128, KLOC], f32)
        nc.sync.dma_start(cm[:], cm_d[:])
        gvl = const.tile([128, KLOC], f32)
        nc.sync.dma_start(gvl[:], gvl_d[:])
        cg = const.tile([128, KLOC], f32)
        nc.sync.dma_start(cg[:], cg_d[:])

        # v in [batch, k] layout, one tile per batch subtile
        vbk = []
        for bs in range(NBS):
            t = vbk_pool.tile([128, KLOC], f32, tag=f"vbk{bs}", name=f"vbk{bs}")
            nc.sync.dma_start(t[:], hxbk_d[bs * 128:(bs + 1) * 128, :])
            vbk.append(t)

        # ---- sensory pass (once) ----
        # psum [128 batch, 4 bsub-quadrant, 64 k, 2 (num,den)] x2 banks
        pss = [ps_sens.tile([128, 4, KLOC, 2], f32, name=f"pss{i}")
               for i in range(2)]
        for k in range(KLOC):
            sg = s_pool.tile([I, B], f16, tag="ssens", name=f"ssens{k}")
            nc.scalar.activation(sg[:], xT[:], AF.Sigmoid,
                                 bias=sbi[:, k:k + 1], scale=ssc[:, k:k + 1])
            for bs in range(NBS):
                nc.tensor.matmul(
                    pss[bs // 4][:, bs % 4, k, :],
                    lhsT=sg[:, bs * 128:(bs + 1) * 128],
                    rhs=swp[:, 2 * k:2 * k + 2],
                    start=True, stop=True)
        # fold gleak*vleak and cm+gleak into the sensory sums -> SBUF
        sens_num, sens_den = [], []
        for bs in range(NBS):
            sn = sens_pool.tile([128, KLOC], f32, tag=f"sn{bs}", name=f"sn{bs}")
            nc.vector.tensor_add(sn[:], pss[bs // 4][:, bs % 4, :, 0], gvl[:])
            sd = sens_pool.tile([128, KLOC], f32, tag=f"sd{bs}", name=f"sd{bs}")
            nc.vector.tensor_add(sd[:], pss[bs // 4][:, bs % 4, :, 1], cg[:])
            sens_num.append(sn)
            sens_den.append(sd)

        # ---- unfolds ----
        # The presynaptic (j) axis is PERMUTED host-side so that new row
        # r = chunk*256 + core*32 + kappa holds old j = core*64 + chunk*32
        # + kappa.  Then the AllGather of chunk c (every core's local k in
        # [c*32, c*32+32)) is, verbatim, new rows [c*256, c*256+256) — i.e.
        # j-tiles {0,1} come from collective 0 and j-tiles {2,3} from
        # collective 1.  Per unfold the ACT work is ordered so that
        #   phase A  (all k,  jt 0,1)  needs only the previous collective 0,
        #   phase B1 (k<32,  jt 2,3)  completes chunk-0 psums,
        #   -> chunk-0 update + collective 0 issued here,
        #   phase B2 (k>=32, jt 2,3)  hides collective 0,
        #   -> chunk-1 update + collective 1, hidden under the next
        #      unfold's phase A.
        # In steady state the ACT engine never waits on a gather.
        HALF = KLOC // 2

        # vt tiles for unfold 0 come straight from (permuted) hxT.
        vt = []
        for jt in range(NJT):
            t = vt_pool.tile([128, B], f16, tag=f"vt{jt}", name=f"vt0_{jt}")
            nc.sync.dma_start(t[:], hxT_d[jt * 128:(jt + 1) * 128, :])
            vt.append(t)

        for u in range(unfolds):
            psr = [ps_rec.tile([128, 4, KLOC, 2], f32, tag=f"psr{i}",
                               name=f"psr{u}_{i}") for i in range(2)]
            last = u == unfolds - 1
            if not last:
                vloc = vloc_pool.tile([KLOC, B], f16, tag="vloc",
                                      name=f"vloc{u}")
                vt_next = [vt_pool.tile([128, B], f16, tag=f"vt{jt}",
                                        name=f"vt{u + 1}_{jt}")
                           for jt in range(NJT)]
            new_vbk = [vbk_pool.tile([128, KLOC], f32, tag=f"vbk{bs}",
                                     name=f"vbk{u}_{bs}")
                       for bs in range(NBS)]

            def act_mm(k, jt, u=u, vt=vt, psr=psr):
                col = jt * KLOC + k
                sg = s_pool.tile([128, B], f16, tag=f"s{jt}",
                                 name=f"s{u}_{k}_{jt}")
                nc.scalar.activation(sg[:], vt[jt][:], AF.Sigmoid,
                                     bias=rbi[:, col:col + 1],
                                     scale=rsc[:, col:col + 1])
                for bs in range(NBS):
                    # start=True clears has_written for the WHOLE psum
                    # bank, so only the first matmul into each bank per
                    # unfold may carry it; later writers then overwrite
                    # (bit clear) or accumulate (bit set) per element.
                    nc.tensor.matmul(
                        psr[bs // 4][:, bs % 4, k, :],
                        lhsT=sg[:, bs * 128:(bs + 1) * 128],
                        rhs=rwp[:, 2 * col:2 * col + 2],
                        start=(k == 0 and jt == 0 and bs % 4 == 0),
                        stop=(k == KLOC - 1 and jt == NJT - 1
                              and bs % 4 == 3),
                        skip_group_check=True)

            def chunk_epilogue(ci, u=u, psr=psr, new_vbk=new_vbk, vbk=vbk,
                               last=last):
                k0, k1 = ci * HALF, (ci + 1) * HALF
                # update: v_new = (cm*v + num + sens_num) / (den + sens_den)
                for bs in range(NBS):
                    n1 = upd_pool.tile([128, HALF], f32, tag=f"n{bs}",
                                       name=f"n{u}_{ci}_{bs}")
                    nc.vector.tensor_mul(n1[:], vbk[bs][:, k0:k1],
                                         cm[:, k0:k1])
                    nc.vector.tensor_add(n1[:], n1[:],
                                         psr[bs // 4][:, bs % 4, k0:k1, 0])
                    nc.vector.tensor_add(n1[:], n1[:], sens_num[bs][:, k0:k1])
                    d1 = upd_pool.tile([128, HALF], f32, tag=f"d{bs}",
                                       name=f"d{u}_{ci}_{bs}")
                    nc.vector.tensor_add(d1[:],
                                         psr[bs // 4][:, bs % 4, k0:k1, 1],
                                         sens_den[bs][:, k0:k1])
                    nc.vector.reciprocal(d1[:], d1[:])
                    nc.vector.tensor_mul(new_vbk[bs][:, k0:k1], n1[:], d1[:])
                if last:
                    return
                # transpose chunk to [k, b] (PE transpose into a chunk-sized
                # PSUM tile at partition 0), cast-copy into the f16 gather
                # staging tile, gather across cores, and load the next
                # unfold's two j-tiles.
                pt = [ps_tr.tile([HALF, 4, 128], f32, tag=f"pt{i}",
                                 name=f"pt{u}_{ci}_{i}") for i in range(2)]
                for bs in range(NBS):
                    nc.tensor.transpose(pt[bs // 4][:, bs % 4, :],
                                        new_vbk[bs][:, k0:k1], ident[:])
                    nc.vector.tensor_copy(
                        vloc[k0:k1, bs * 128:(bs + 1) * 128],
                        pt[bs // 4][:, bs % 4, :])
                g_in = dram.tile([HALF, B], f16, tag=f"gin{ci}",
                                 name=f"gin{u}_{ci}")
                g_out = dram.tile([NCORES * HALF, B], f16, tag=f"gout{ci}",
                                  name=f"gout{u}_{ci}", addr_space="Shared")
                nc.sync.dma_start(g_in[:], vloc[k0:k1, :])
                nc.gpsimd.collective_compute(
                    "AllGather", mybir.AluOpType.bypass,
                    replica_groups=[list(range(NCORES))],
                    ins=[g_in.opt()], outs=[g_out.opt()])
                # g_out row c*32+kappa = new j row ci*256 + c*32 + kappa.
                for h in range(2):
                    jt = 2 * ci + h
                    nc.sync.dma_start(vt_next[jt][:],
                                      g_out[h * 128:(h + 1) * 128, :])

            for k in range(KLOC):            # phase A
                for jt in (0, 1):
                    act_mm(k, jt)
            for k in range(HALF):            # phase B1
                for jt in (2, 3):
                    act_mm(k, jt)
            chunk_epilogue(0)
            for k in range(HALF, KLOC):      # phase B2
                for jt in (2, 3):
                    act_mm(k, jt)
            chunk_epilogue(1)

            if last:
                for bs in range(NBS):
                    nc.sync.dma_start(out_d[bs * 128:(bs + 1) * 128, :],
                                      new_vbk[bs][:])
            else:
                vt = vt_next
            vbk = new_vbk

    nc.compile()
    return nc


# Which raw reference inputs each packed device tensor depends on.
_PACK_DEPS = {
    "xT": ("inputs", "input_w", "input_b"),
    "hxT": ("hx",),
    "hxbk": ("hx",),
    "rsc": ("sigma",),
    "rbi": ("sigma", "mu"),
    "rwp": ("W", "erev"),
    "ssc": ("sensory_sigma",),
    "sbi": ("sensory_sigma", "sensory_mu"),
    "swp": ("sensory_W", "sensory_erev"),
    "cmr": ("cm_t",),
    "gvlr": ("gleak", "vleak"),
    "cgr": ("cm_t", "gleak"),
    "ident": (),
}


def _pack_inputs(inputs, hx, input_w, input_b, sensory_mu, sensory_sigma,
                 sensory_W, sensory_erev, mu, sigma, W, erev, vleak, gleak,
                 cm_t):
    """Host-side repack: per-core parameter slices + transposed state."""
    f32 = np.float32
    f16 = np.float16
    x = (inputs * input_w + input_b).astype(f32)
    xT = np.ascontiguousarray(x.T)                       # [I, B]
    # presynaptic permutation: new row r = chunk*256 + core*32 + kappa
    # holds old j = core*64 + chunk*32 + kappa (see _build_program).
    r = np.arange(S)
    perm = (r % 256) // 32 * KLOC + (r // 256) * 32 + r % 32
    hxT = np.ascontiguousarray(hx.T[perm].astype(f16))   # [S, B] permuted
    sigma = sigma[perm, :]
    mu = mu[perm, :]
    W = W[perm, :]
    erev = erev[perm, :]
    neg_d = -(sigma * mu)
    sneg_d = -(sensory_sigma * sensory_mu)
    Werev = W * erev
    sWerev = sensory_W * sensory_erev
    gvl = (gleak * vleak).astype(f32)
    cg = (cm_t + gleak).astype(f32)

    def pack_jt_k(a, ks):                                # [S, S] -> [128, 4*64]
        return np.ascontiguousarray(
            a.reshape(NJT, 128, S)[:, :, ks].transpose(1, 0, 2).reshape(128, NJT * KLOC))

    def pack_pairs(a, b, ks):                            # -> [128, 4*64*2]
        st = np.stack([a, b], axis=-1)                   # [S, S, 2]
        return np.ascontiguousarray(
            st.reshape(NJT, 128, S, 2)[:, :, ks, :].transpose(1, 0, 2, 3)
            .reshape(128, NJT * KLOC * 2))

    in_maps = []
    for c in range(NCORES):
        ks = slice(c * KLOC, (c + 1) * KLOC)
        m = {
            "xT": xT,
            "hxT": hxT,
            "hxbk": np.ascontiguousarray(hx[:, ks].astype(f32)),
            "rsc": pack_jt_k(sigma.astype(f32), ks),
            "rbi": pack_jt_k(neg_d.astype(f32), ks),
            "rwp": pack_pairs(Werev, W, ks).astype(f16),
            "ssc": np.ascontiguousarray(sensory_sigma[:, ks].astype(f32)),
            "sbi": np.ascontiguousarray(sneg_d[:, ks].astype(f32)),
            "swp": np.ascontiguousarray(
                np.stack([sWerev[:, ks], sensory_W[:, ks]], axis=-1)
                .reshape(I, KLOC * 2)).astype(f16),
            "cmr": np.ascontiguousarray(
                np.broadcast_to(cm_t[ks].astype(f32), (128, KLOC))),
            "gvlr": np.ascontiguousarray(
                np.broadcast_to(gvl[ks], (128, KLOC))),
            "cgr": np.ascontiguousarray(
                np.broadcast_to(cg[ks], (128, KLOC))),
            "ident": np.eye(128, dtype=f32),
        }
        in_maps.append(m)
    return in_maps


class _Runner:
    """One-time-compiled SPMD executable with device-resident input cache."""

    def __init__(self, nc):
        import jax
        from concourse import bass2jax, mybir
        from jax.experimental.shard_map import shard_map
        from jax.sharding import Mesh, NamedSharding, PartitionSpec

        self.jax = jax
        self.np = np
        bass2jax.install_neuronx_cc_hook()
        self.nc = nc

        partition_name = (nc.partition_id_tensor.name
                          if nc.partition_id_tensor else None)
        in_names, out_names, out_avals, zero_shapes = [], [], [], []
        for alloc in nc.m.functions[0].allocations:
            if not isinstance(alloc, mybir.MemoryLocationSet):
                continue
            name = alloc.memorylocations[0].name
            if alloc.kind == "ExternalInput":
                if name != partition_name:
                    in_names.append(name)
            elif alloc.kind == "ExternalOutput":
                shape = tuple(alloc.tensor_shape)
                dtype = mybir.dt.np(alloc.dtype)
                out_names.append(name)
                out_avals.append(jax.core.ShapedArray(shape, dtype))
                zero_shapes.append((shape, dtype))
        n_params = len(in_names)
        n_outs = len(out_names)
        all_in_names = list(in_names) + list(out_names)
        if partition_name is not None:
            all_in_names.append(partition_name)
        self.in_names = in_names
        self.out_names = out_names
        donate = tuple(range(n_params, n_params + n_outs))

        dbg_zero = None
        if nc.dbg_addr is not None:
            if nc.dbg_callbacks:
                raise RuntimeError("dbg_callbacks unsupported in this runner")
            dbg_zero = np.zeros((1, 2), np.uint32)
        self.dbg_zero = dbg_zero

        def _body(*args):
            operands = list(args)
            if partition_name is not None:
                operands.append(bass2jax.partition_id_tensor())
            outs = bass2jax._bass_exec_p.bind(
                *operands,
                out_avals=tuple(out_avals),
                in_names=tuple(all_in_names),
                out_names=tuple(out_names),
                lowering_input_output_aliases=(),
                sim_require_finite=True,
                sim_require_nnan=True,
                nc=nc,
            )
            return tuple(outs)

        devices = jax.devices()[:NCORES]
        assert len(devices) == NCORES
        mesh = Mesh(np.asarray(devices), ("core",))
        self.sharding = NamedSharding(mesh, PartitionSpec("core"))
        in_specs = (PartitionSpec("core"),) * (n_params + n_outs)
        out_specs = (PartitionSpec("core"),) * n_outs
        fn = shard_map(_body, mesh=mesh, in_specs=in_specs,
                       out_specs=out_specs, check_rep=False)

        # AOT-compile once; prefer the effect-free C++ fast-dispatch path.
        arg_structs = []
        sample_names = self.in_names + list(range(n_outs))

        self._arg_struct = None  # filled on first inputs (needs shapes)
        self._fn = fn
        self._donate = donate
        self._jit = jax.jit(fn, donate_argnums=donate, keep_unused=True)
        self._compiled = None
        self._donor = None  # previous output buffer, donated as out-init

        self.zero_shapes = zero_shapes

    def _global(self, arr):
        """Per-core array -> committed global [8*d0, ...] device array."""
        jax = self.jax
        g = np.concatenate([arr] * NCORES, axis=0) if arr.ndim else arr
        return jax.device_put(g, self.sharding)

    def upload_concat(self, per_core_list):
        g = np.concatenate(per_core_list, axis=0)
        return self.jax.device_put(g, self.sharding)

    def _ensure_compiled(self, args):
        if self._compiled is not None:
            return
        jax = self.jax
        from concourse import bass2jax
        structs = [jax.ShapeDtypeStruct(a.shape, a.dtype, sharding=a.sharding)
                   for a in args]

        def _do_compile():
            jitted = jax.jit(self._fn, donate_argnums=self._donate,
                             keep_unused=True)
            return jitted.lower(*structs).compile()

        try:
            self._compiled = bass2jax.fast_dispatch_compile(_do_compile)
        except Exception:
            self._compiled = _do_compile()

    def execute(self, in_bufs):
        """One async SPMD execution; returns the (unfetched) output array."""
        jax = self.jax
        if self._donor is None:
            donor = jax.device_put(
                np.zeros((NCORES * self.zero_shapes[0][0][0],)
                         + tuple(self.zero_shapes[0][0][1:]),
                         self.zero_shapes[0][1]),
                self.sharding)
        else:
            donor = self._donor
        args = list(in_bufs) + [donor]
        self._ensure_compiled(args)
        out = self._compiled(*args)
        out = out[0] if isinstance(out, (tuple, list)) else out
        self._donor = out
        return out


def _ensure_runner():
    if "runner" not in _CACHE:
        nc = _CACHE.get("nc")
        if nc is None:
            nc = _build_program()
            _CACHE["nc"] = nc
        _CACHE["runner"] = _Runner(nc)
    return _CACHE["runner"]


def _ensure_inputs(inputs_dict):
    """Return the device input buffers, reusing cached uploads when the raw
    inputs' content is unchanged."""
    r = _ensure_runner()
    # copy: the cache must not alias caller buffers (in-place mutation by
    # the caller would otherwise defeat the change detection)
    raw = {k: np.array(v, copy=True) for k, v in inputs_dict.items()}
    cached_raw = _CACHE.get("raw")
    changed_raw = set(raw)
    if cached_raw is not None:
        changed_raw = {k for k, v in raw.items()
                       if not np.array_equal(v, cached_raw[k])}
    if cached_raw is None or changed_raw:
        in_maps = _pack_inputs(**raw)
        packed = {name: [m[name] for m in in_maps] for name in _PACK_DEPS}
        bufs = dict(_CACHE.get("bufs", {}))
        old_packed = _CACHE.get("packed")
        for name, deps in _PACK_DEPS.items():
            stale = (cached_raw is None or name not in bufs
                     or any(d in changed_raw for d in deps))
            if stale:
                if (old_packed is not None and name in bufs and
                        all(np.array_equal(a, b) for a, b in
                            zip(packed[name], old_packed[name]))):
                    continue  # content identical after all
                bufs[name] = r.upload_concat(packed[name])
        _CACHE["raw"] = raw
        _CACHE["packed"] = packed
        _CACHE["bufs"] = bufs
    return [_CACHE["bufs"][name] for name in r.in_names]


def _unpack_out(out_arr):
    a = np.asarray(out_arr)                       # [8*1024, 64]
    a = a.reshape(NCORES, B, KLOC).transpose(1, 0, 2).reshape(B, S)
    return np.ascontiguousarray(a, dtype=np.float32)


def kernel(**inputs):
    r = _ensure_runner()
    bufs = _ensure_inputs(inputs)
    out = r.execute(bufs)
    return _unpack_out(out)
